# revision 1
# baseline (speedup 1.0000x reference)
"""Trainium2 Bass kernel for nn_BaseLUTLayer (soft-LUT layer).

Math: out[b,o] = sum_k lut[o,k] * prod_j (bit_j(k) ? x[b,m(o,j)] : 1-x[b,m(o,j)])

Strategy (per core, batch-sharded 8 ways, 128 batch rows each):
  * odds transform: with w = 1-x, r = x/(1-x):
        out[b,o] = (prod_j w_j) * H,   H = successive halving of lut with
        T_new[k'] = T_lo[k'] + r_j * T_hi[k']   (6 levels, 2 DVE ops/elem)
  * layout: nodes on SBUF partitions (o_p = o % 128), free dims (k', b).
    lut tiles live per-partition (no replication); r/w values are gathered
    per (node, wire) with dma_gather using compile-time indices derived
    from `mapping` (host-known at trace time).
  * gather source: G[row(i)] = [w[:,i] (128 f32) | r[:,i] (128 f32)] built
    on-device (clamp, 1-x, reciprocal, PE transposes) and bounced via HBM.
"""

import numpy as np

import concourse.bass as bass
import concourse.mybir as mybir
from concourse import bacc
from concourse import tile
from concourse.masks import make_identity
from concourse.bass_utils import run_bass_kernel_spmd

P = 128
IN = 1024
OUT = 2048
NB = 6
B_FULL = 1024
N_CORES = 8
OHI = OUT // P  # 16
F32 = mybir.dt.float32
I16 = mybir.dt.int16
# clamp x <= 1 - 2^-18 so r = x/(1-x) <= 2^18 and r^6 stays well inside fp32
CLAMP = float(1.0 - 2.0**-18)

# chunks of the o_hi loop assigned to gpsimd instead of DVE (load balance:
# gpsimd 2-input elementwise is ~2x slower than DVE, so give it ~1/3)
GPSIMD_CHUNKS = ()
K_ACT = 28  # level-1 k'-slices on ScalarE; rest on DVE


def _mult():
    return mybir.AluOpType.mult


def _add():
    return mybir.AluOpType.add


def build_program():
    nc = bacc.Bacc("TRN2", target_bir_lowering=False, debug=False)

    xs = nc.dram_tensor("xs", [P, IN], F32, kind="ExternalInput").ap()
    gidx = nc.dram_tensor("gidx", [P, OUT * NB // 16], I16, kind="ExternalInput").ap()
    lutg = nc.dram_tensor("lutg", [P, OHI, 64], F32, kind="ExternalInput").ap()
    outs = nc.dram_tensor("outs", [P, OHI, P], F32, kind="ExternalOutput").ap()

    with tile.TileContext(nc) as tc:
        with (
            tc.tile_pool(name="consts", bufs=1) as consts,
            tc.tile_pool(name="main", bufs=1) as main,
            tc.tile_pool(name="zpool", bufs=5) as zpool,
            tc.tile_pool(name="tpool", bufs=3) as tpool,
            tc.tile_pool(name="spool", bufs=2) as spool,
            tc.tile_pool(name="dram", bufs=1, space="DRAM") as dpool,
        ):
            ident = consts.tile([P, P], F32)
            make_identity(nc, ident)

            gd = dpool.tile([P * (IN // P), 2 * P], F32)
            gd_warm = gd

            gidx_sb = consts.tile([P, OUT * NB // 16], I16)
            nc.sync.dma_start(gidx_sb, gidx)
            lutg_sb = consts.tile([P, OHI, 64], F32)
            nc.sync.dma_start(lutg_sb, lutg)

            # warm up the dma_gather ucode (IRAM load) before G is ready:
            # zero gd row 0, gather it 128 times into a scratch tile
            wzt = consts.tile([1, 2 * P], F32)
            nc.gpsimd.memset(wzt, 0.0)
            nc.sync.dma_start(gd_warm[0:1, :], wzt)
            widx = consts.tile([P, 8], I16)
            nc.gpsimd.memset(widx, 0)
            warm = consts.tile([P, 1, 2 * P], F32)
            nc.gpsimd.dma_gather(
                out_ap=warm,
                in_ap=gd_warm[0:1, :],
                idxs_ap=widx,
                num_idxs=P,
                num_idxs_reg=P,
                elem_size=2 * P,
            )

            # x shard, clamped; w = 1-x; r = x * (1/w)
            xt = main.tile([P, IN], F32)
            nc.sync.dma_start(xt, xs)
            nc.vector.tensor_scalar_min(xt, xt, CLAMP)
            wt = main.tile([P, IN], F32)
            nc.vector.tensor_scalar(
                out=wt, in0=xt, scalar1=-1.0, scalar2=1.0, op0=_mult(), op1=_add()
            )
            rw = main.tile([P, IN], F32)
            rt = main.tile([P, IN], F32)
            for q in range(4):
                qs = slice(q * (IN // 4), (q + 1) * (IN // 4))
                nc.vector.reciprocal(rw[:, qs], wt[:, qs])
                nc.vector.tensor_mul(rt[:, qs], xt[:, qs], rw[:, qs])

            # transpose w/r into G rows: G[(i%128)*8 + i//128] = [w[:,i] | r[:,i]]
            gsb = main.tile([P, IN // P, 2 * P], F32)
            with tc.tile_pool(name="psum_t", bufs=2, space="PSUM") as psum_t:
                for ih in range(IN // P):
                    pw = psum_t.tile([P, P], F32, tag="pt")
                    nc.tensor.transpose(pw, wt[:, ih * P : (ih + 1) * P], ident)
                    nc.scalar.copy(gsb[:, ih, 0:P], pw)
                    pr = psum_t.tile([P, P], F32, tag="pt")
                    nc.tensor.transpose(pr, rt[:, ih * P : (ih + 1) * P], ident)
                    nc.scalar.copy(gsb[:, ih, P : 2 * P], pr)

            gd_view = gd[:].rearrange("(p h) e -> p h e", h=IN // P)
            for ih in range(IN // P):
                nc.sync.dma_start(gd_view[:, ih, :], gsb[:, ih, :])

            # main loop over node chunks (128 nodes each)
            psum_cm = tc.tile_pool(name="psum", bufs=2, space="PSUM")
            psum = psum_cm.__enter__()
            idx_cols = NB * P // 16  # 48 idx columns per chunk

            # two-stage software pipeline: stage A (gather + monomial muls +
            # DMA pair-adds) for chunk c, then stage B (everything after the
            # DMA-adds) for chunk c-1 — keeps DVE's in-order queue from
            # stalling on the DMA-add completion.
            stash = {}

            def stage_a(c):
                z = zpool.tile([P, NB, 2 * P], F32, tag="z")
                if c == 0:
                    # split the first gather so L1/L2 (slots 0-2 = r5,r4,r3)
                    # can start before the whole chunk lands
                    half = idx_cols // 2
                    nc.gpsimd.dma_gather(
                        out_ap=z[:, 0 : NB // 2, :],
                        in_ap=gd[:],
                        idxs_ap=gidx_sb[:, 0:half],
                        num_idxs=NB * P // 2,
                        num_idxs_reg=NB * P // 2,
                        elem_size=2 * P,
                    )
                    nc.gpsimd.dma_gather(
                        out_ap=z[:, NB // 2 : NB, :],
                        in_ap=gd[:],
                        idxs_ap=gidx_sb[:, half:idx_cols],
                        num_idxs=NB * P // 2,
                        num_idxs_reg=NB * P // 2,
                        elem_size=2 * P,
                    )
                else:
                    nc.gpsimd.dma_gather(
                        out_ap=z,
                        in_ap=gd[:],
                        idxs_ap=gidx_sb[:, c * idx_cols : (c + 1) * idx_cols],
                        num_idxs=NB * P,
                        num_idxs_reg=NB * P,
                        elem_size=2 * P,
                    )
                # W = prod_j w_j (DVE)
                wp = spool.tile([P, 3, P], F32, tag="wp")
                nc.vector.tensor_mul(wp, z[:, 0:5:2, 0:P], z[:, 1:6:2, 0:P])
                wq = spool.tile([P, P], F32, tag="wq")
                nc.vector.tensor_mul(wq, wp[:, 0, :], wp[:, 1, :])
                nc.vector.tensor_mul(wq, wq, wp[:, 2, :])
                # level 1 on the (otherwise idle) Scalar engine:
                # t1[:, k', :] = r5 * lut1[k'] + lut0[k']  — lut entries are
                # per-partition scalars (scale/bias), r5 is the tensor input
                t1 = tpool.tile([P, 32, P], F32, tag="t1")
                r5t = z[:, 0, P : 2 * P]
                for kp in range(K_ACT):
                    nc.scalar.activation(
                        t1[:, kp, :],
                        r5t,
                        mybir.ActivationFunctionType.Identity,
                        bias=lutg_sb[:, c, kp : kp + 1],
                        scale=lutg_sb[:, c, 32 + kp : 33 + kp],
                    )
                kd = 32 - K_ACT
                if kd:
                    nc.vector.tensor_mul(
                        t1[:, K_ACT:32, :],
                        r5t[:, None, :].broadcast_to([P, kd, P]),
                        lutg_sb[:, c, 32 + K_ACT : 64][:, :, None].broadcast_to([P, kd, P]),
                    )
                    nc.vector.tensor_add(
                        t1[:, K_ACT:32, :],
                        t1[:, K_ACT:32, :],
                        lutg_sb[:, c, K_ACT:32][:, :, None].broadcast_to([P, kd, P]),
                    )
                # level 2 prod (DVE): prod2 = r4 * T1_hi ; t2 = T1_lo + prod2
                prod2 = tpool.tile([P, 16, P], F32, tag="pr16")
                nc.vector.tensor_mul(
                    prod2,
                    z[:, 1, P : 2 * P][:, None, :].broadcast_to([P, 16, P]),
                    t1[:, 16:32, :],
                )
                t2 = tpool.tile([P, 16, P], F32, tag="t2")
                nc.vector.tensor_add(t2, prod2, t1[:, 0:16, :])
                stash[c] = (z, t2, wq)

            def stage_b1(c):
                z, t2, wq = stash.pop(c)

                # level 3 (j=3, h=8): prod3 = r3*t2_hi (DVE);
                # acc[0:1024] = t2_lo + prod3 on TensorE; close R1 only
                pn8 = tpool.tile([P, 8, P], F32, tag="pr8")
                nc.vector.tensor_mul(
                    pn8,
                    z[:, 2, P : 2 * P][:, None, :].broadcast_to([P, 8, P]),
                    t2[:, 8:16, :],
                )
                t2f = t2[:].rearrange("p a b -> p (a b)")
                pn8f = pn8[:].rearrange("p a b -> p (a b)")
                acc = psum.tile([P, 8 * P], F32, tag="pacc")
                for s in range(2):
                    sl = slice(s * 512, (s + 1) * 512)
                    nc.tensor.matmul(
                        acc[:, sl], ident, t2f[:, sl], start=True, stop=False
                    )
                    nc.tensor.matmul(
                        acc[:, sl], ident, pn8f[:, sl], start=False, stop=(s == 1)
                    )
                stash[(c, "b2")] = (z, acc, wq)

            def stage_b2(c):
                z, acc, wq = stash.pop((c, "b2"))

                # level 4 (j=2, h=4): prod4 = r2*acc[4:8] (DVE);
                # acc[0:4] += prod4 (PE), closing R0
                pn4 = tpool.tile([P, 4, P], F32, tag="pr4")
                nc.vector.tensor_mul(
                    pn4,
                    z[:, 3, P : 2 * P][:, None, :].broadcast_to([P, 4, P]),
                    acc[:, 4 * P : 8 * P].rearrange("p (a b) -> p a b", b=P),
                )
                pn4f = pn4[:].rearrange("p a b -> p (a b)")
                nc.tensor.matmul(
                    acc[:, 0:512], ident, pn4f[:, 0:512], start=False, stop=True
                )

                # level 5 (j=1, h=2) from closed PSUM
                pn2 = tpool.tile([P, 2, P], F32, tag="pr2")
                nc.vector.tensor_mul(
                    pn2,
                    z[:, 4, P : 2 * P][:, None, :].broadcast_to([P, 2, P]),
                    acc[:, 2 * P : 4 * P].rearrange("p (a b) -> p a b", b=P),
                )
                t5 = tpool.tile([P, 2, P], F32, tag="t5")
                nc.vector.tensor_add(
                    t5, pn2, acc[:, 0 : 2 * P].rearrange("p (a b) -> p a b", b=P)
                )

                # level 6 (j=0, h=1)
                pn1 = tpool.tile([P, 1, P], F32, tag="pr1")
                nc.vector.tensor_mul(
                    pn1,
                    z[:, 5, P : 2 * P][:, None, :].broadcast_to([P, 1, P]),
                    t5[:, 1:2, :],
                )
                t6 = tpool.tile([P, 1, P], F32, tag="t6")
                nc.vector.tensor_add(t6, pn1, t5[:, 0:1, :])

                ot = spool.tile([P, P], F32, tag="ot")
                nc.vector.tensor_mul(ot, t6[:, 0, :], wq)
                nc.sync.dma_start(outs[:, c, :], ot)

            for c in range(OHI + 1):
                if c < OHI:
                    stage_a(c)
                if c >= 1:
                    stage_b1(c - 1)
                    stage_b2(c - 1)
            psum_cm.__exit__(None, None, None)

    # Bacc passes: event-sem generation (multi-wait lowering), auto library
    # loads for dma_gather, extended-InstISA byte packing, ...
    nc.compile()
    return nc


_CACHE: dict = {}


def _program():
    if "nc" not in _CACHE:
        _CACHE["nc"] = build_program()
    return _CACHE["nc"]


def make_inputs(x, lut_table, mapping):
    """Host-side input prep: shard x by batch, encode mapping as gather
    indices, split lut into node-on-partition lo/hi tiles."""
    x = np.ascontiguousarray(x, dtype=np.float32)
    lut_table = np.ascontiguousarray(lut_table, dtype=np.float32)
    mapping = np.asarray(mapping)

    # gather row of source column i: G row (i%128)*8 + i//128
    m3 = mapping.reshape(OHI, P, NB)  # [o_hi, o_p, j]
    rows = (m3 % P) * (IN // P) + (m3 // P)
    # t = (o_hi*NB + slot)*128 + o_p with slot = 5-j  ->  order (o_hi, 5-j, o_p)
    tvals = np.transpose(rows[:, :, ::-1], (0, 2, 1)).reshape(-1)
    gidx16 = tvals.reshape(-1, 16).T.astype(np.int16)  # [16, OUT*NB/16]
    gidx_arr = np.ascontiguousarray(np.tile(gidx16, (P // 16, 1)))

    lut3 = lut_table.reshape(OHI, P, 64).transpose(1, 0, 2)  # [o_p, o_hi, 64]
    lutg_arr = np.ascontiguousarray(lut3)

    in_maps = []
    for core in range(N_CORES):
        in_maps.append(
            {
                "xs": np.ascontiguousarray(x[core * P : (core + 1) * P]),
                "gidx": gidx_arr,
                "lutg": lutg_arr,
            }
        )
    return in_maps


def assemble_output(results):
    """results: list of 8 dicts with 'outs' [128, 16, 128] -> full [1024, 2048]."""
    parts = []
    for core in range(N_CORES):
        arr = results[core]["outs"]  # [o_p, o_hi, b]
        parts.append(np.ascontiguousarray(arr.transpose(2, 1, 0).reshape(P, OUT)))
    return np.concatenate(parts, axis=0)


def kernel_with_results(x, lut_table, mapping, **kwargs):
    nc = _program()
    in_maps = make_inputs(x, lut_table, mapping)
    res = run_bass_kernel_spmd(nc, in_maps, core_ids=list(range(N_CORES)), **kwargs)
    return assemble_output(res.results), res


def kernel(x, lut_table, mapping):
    out, _ = kernel_with_results(x, lut_table, mapping)
    return out


if __name__ == "__main__":
    rng = np.random.default_rng(0)
    x = rng.random((B_FULL, IN), dtype=np.float32)
    lut = rng.standard_normal((OUT, 64), dtype=np.float32)
    mp = rng.integers(0, IN, (OUT, NB), dtype=np.int32)
    out = kernel(x, lut, mp)
    print(out.shape, out.dtype)



# revision 3
# speedup vs baseline: 1.0858x; 1.0858x over previous
"""Trainium2 Bass kernel for nn_BaseLUTLayer (soft-LUT layer).

Math: out[b,o] = sum_k lut[o,k] * prod_j (bit_j(k) ? x[b,m(o,j)] : 1-x[b,m(o,j)])

Strategy (node-sharded 8 ways: each core owns 256 nodes x full batch 1024):
  * odds transform: with w = 1-x, r = x/(1-x) = 1/w - 1:
        out[b,o] = (prod_j w_j) * H,   H = successive halving of lut with
        T_new[k'] = T_lo[k'] + r_j * T_hi[k']   (6 levels)
  * bf16 pipeline: DVE tensor_tensor bf16 runs 2x fp32; tensor_scalar 4x.
  * level 1 is 32 tensor_scalar slices (lut entries are per-partition
    scalars), split across DVE / ScalarE / GpSimd.
  * layout: nodes on SBUF partitions (2 chunks of 128), free dim = b=1024.
    Host ships xT so the gather source table G[wire] = [w|r] needs no
    on-device transposes; G is bounced via DRAM and rows are fetched with
    dma_gather (4KB bf16 rows, split over 2 SWDGE queues).
  * halving runs in place inside t1 (t_k = t1[:, 0:2^(5-k), :]) to fit SBUF.
"""

import numpy as np

import concourse.bass as bass
import concourse.mybir as mybir
from concourse import bacc
from concourse import tile
from concourse.bass_utils import run_bass_kernel_spmd

P = 128
IN = 1024
OUT = 2048
NB = 6
B_FULL = 1024
N_CORES = 8
NODES_PER_CORE = OUT // N_CORES  # 256
NCHUNK = NODES_PER_CORE // P  # 2
B = B_FULL  # free dim per core
F32 = mybir.dt.float32
BF16 = mybir.dt.bfloat16
I16 = mybir.dt.int16
MULT = mybir.AluOpType.mult
ADD = mybir.AluOpType.add
# clamp x <= 1 - 2^-18 so r = x/(1-x) <= 2^18; bf16 range is fp32-like
CLAMP = float(1.0 - 2.0**-18)

# L1 slice split: [0, DVE_K) on DVE, [DVE_K, DVE_K+SC_K) on ScalarE, rest gpsimd
DVE_K = 18
SC_K = 10


def build_program():
    nc = bacc.Bacc(
        "TRN2", target_bir_lowering=False, debug=False, num_swdge_queues=2
    )

    xt = nc.dram_tensor("xt", [P, IN // P, B], F32, kind="ExternalInput").ap()
    gidx = nc.dram_tensor(
        "gidx", [P, NCHUNK * NB * P // 16], I16, kind="ExternalInput"
    ).ap()
    lutg = nc.dram_tensor("lutg", [P, NCHUNK, 64], F32, kind="ExternalInput").ap()
    outs = nc.dram_tensor("outs", [P, NCHUNK, B], F32, kind="ExternalOutput").ap()

    IH = IN // P  # 8

    with tile.TileContext(nc) as tc:
        with (
            tc.tile_pool(name="consts", bufs=1) as consts,
            tc.tile_pool(name="dram", bufs=1, space="DRAM") as dpool,
        ):
            gidx_sb = consts.tile([P, NCHUNK * NB * P // 16], I16)
            nc.sync.dma_start(gidx_sb, gidx)
            lutg_sb = consts.tile([P, NCHUNK, 64], F32)
            nc.sync.dma_start(lutg_sb, lutg)

            # ---- prologue: build G[wire] = [w bf16 | r bf16] rows in DRAM
            gd = dpool.tile([IN, 2 * B], BF16)
            gd_view = gd[:].rearrange("(p h) e -> p h e", h=IH)

            with tc.tile_pool(name="pro", bufs=1) as pro:
                xts = pro.tile([P, IH, B], F32)
                nc.sync.dma_start(xts, xt)
                gsb = pro.tile([P, IH, 2 * B], BF16)
                wf = pro.tile([P, IH // 2, B], F32)
                rf = pro.tile([P, IH // 2, B], F32)
                for h in range(2):
                    hs = slice(h * (IH // 2), (h + 1) * (IH // 2))
                    nc.vector.tensor_scalar_min(xts[:, hs, :], xts[:, hs, :], CLAMP)
                    # w = 1 - x (fp32 for the reciprocal bit-trick)
                    nc.vector.tensor_scalar(
                        out=wf, in0=xts[:, hs, :],
                        scalar1=-1.0, scalar2=1.0, op0=MULT, op1=ADD,
                    )
                    # G w-half (bf16 cast)
                    nc.vector.tensor_copy(gsb[:, hs, 0:B], wf)
                    nc.vector.reciprocal_approx_fast(rf, wf)
                    # r = 1/w - 1  (bf16 out)
                    nc.vector.tensor_scalar_add(gsb[:, hs, B : 2 * B], rf, -1.0)
                    nc.sync.dma_start(gd_view[:, hs, :], gsb[:, hs, :])

            idx_cols = NB * P // 16  # 48 idx columns per chunk

            with (
                tc.tile_pool(name="zpool", bufs=2) as zpool,
                tc.tile_pool(name="tpool", bufs=1) as tpool,
                tc.tile_pool(name="spool", bufs=2) as spool,
            ):

                def gather(c):
                    z = zpool.tile([P, NB, 2 * B], BF16, tag="z")
                    c0 = c * idx_cols
                    nc.gpsimd.dma_gather(
                        out_ap=z[:, 0 : NB // 2, :],
                        in_ap=gd[:],
                        idxs_ap=gidx_sb[:, c0 : c0 + idx_cols // 2],
                        num_idxs=NB * P // 2,
                        num_idxs_reg=NB * P // 2,
                        elem_size=2 * B,
                        queue_num=0,
                    )
                    nc.gpsimd.dma_gather(
                        out_ap=z[:, NB // 2 : NB, :],
                        in_ap=gd[:],
                        idxs_ap=gidx_sb[:, c0 + idx_cols // 2 : c0 + idx_cols],
                        num_idxs=NB * P // 2,
                        num_idxs_reg=NB * P // 2,
                        elem_size=2 * B,
                        queue_num=1,
                    )
                    return z

                def bcast(r, n):
                    return r[:, None, :].broadcast_to([P, n, B])

                def compute(c, z):
                    # W = prod_j w_j on gpsimd (idle after the gathers)
                    wp = spool.tile([P, 3, B], BF16, tag="wp")
                    nc.gpsimd.tensor_mul(wp, z[:, 0:5:2, 0:B], z[:, 1:6:2, 0:B])
                    wq = spool.tile([P, B], BF16, tag="wq")
                    nc.gpsimd.tensor_mul(wq, wp[:, 0, :], wp[:, 1, :])
                    nc.gpsimd.tensor_mul(wq, wq, wp[:, 2, :])

                    r5 = z[:, 0, B : 2 * B]
                    r4 = z[:, 1, B : 2 * B]
                    r3 = z[:, 2, B : 2 * B]
                    r2 = z[:, 3, B : 2 * B]
                    r1 = z[:, 4, B : 2 * B]
                    r0 = z[:, 5, B : 2 * B]

                    # L1: t1[k'] = lut[k'] + r5 * lut[32+k']  (32 slices)
                    t1 = tpool.tile([P, 32, B], BF16, tag="t1")
                    for kp in range(32):
                        lo = lutg_sb[:, c, kp : kp + 1]
                        hi = lutg_sb[:, c, 32 + kp : 33 + kp]
                        if kp < DVE_K:
                            nc.vector.tensor_scalar(
                                out=t1[:, kp, :], in0=r5,
                                scalar1=hi, scalar2=lo, op0=MULT, op1=ADD,
                            )
                        elif kp < DVE_K + SC_K:
                            nc.scalar.activation(
                                t1[:, kp, :], r5,
                                mybir.ActivationFunctionType.Identity,
                                bias=lo, scale=hi,
                            )
                        else:
                            nc.gpsimd.tensor_scalar(
                                out=t1[:, kp, :], in0=r5,
                                scalar1=hi, scalar2=lo, op0=MULT, op1=ADD,
                            )

                    pr = tpool.tile([P, 16, B], BF16, tag="pr")

                    # L2..L6: t_new = t_lo + r_j * t_hi, in place in t1
                    nc.vector.tensor_mul(pr, bcast(r4, 16), t1[:, 16:32, :])
                    nc.vector.tensor_add(t1[:, 0:16, :], pr, t1[:, 0:16, :])

                    nc.vector.tensor_mul(pr[:, 0:8, :], bcast(r3, 8), t1[:, 8:16, :])
                    nc.vector.tensor_add(t1[:, 0:8, :], pr[:, 0:8, :], t1[:, 0:8, :])

                    nc.vector.tensor_mul(pr[:, 0:4, :], bcast(r2, 4), t1[:, 4:8, :])
                    nc.vector.tensor_add(t1[:, 0:4, :], pr[:, 0:4, :], t1[:, 0:4, :])

                    nc.gpsimd.tensor_mul(pr[:, 0:2, :], bcast(r1, 2), t1[:, 2:4, :])
                    nc.gpsimd.tensor_add(t1[:, 0:2, :], pr[:, 0:2, :], t1[:, 0:2, :])

                    nc.gpsimd.tensor_mul(pr[:, 0:1, :], bcast(r0, 1), t1[:, 1:2, :])
                    nc.gpsimd.tensor_add(t1[:, 0:1, :], pr[:, 0:1, :], t1[:, 0:1, :])

                    # final: out = W * t6 (fp32 out)
                    ot = spool.tile([P, B], F32, tag="ot")
                    nc.vector.tensor_mul(ot, t1[:, 0, :], wq)
                    nc.sync.dma_start(outs[:, c, :], ot)

                zs = {}
                zs[0] = gather(0)
                for c in range(NCHUNK):
                    if c + 1 < NCHUNK:
                        zs[c + 1] = gather(c + 1)
                    compute(c, zs.pop(c))

    nc.compile()
    return nc


_CACHE: dict = {}


def _program():
    if "nc" not in _CACHE:
        _CACHE["nc"] = build_program()
    return _CACHE["nc"]


def make_inputs(x, lut_table, mapping):
    """Host-side input prep: transpose x (layout only), per-core node shards
    of lut and gather indices."""
    x = np.ascontiguousarray(x, dtype=np.float32)
    lut_table = np.ascontiguousarray(lut_table, dtype=np.float32)
    mapping = np.asarray(mapping)

    # xt[p, h, b] = x[b, p*8+h]  (wire i = p*8+h -> G row i)
    xt_arr = np.ascontiguousarray(x.T.reshape(P, IN // P, B_FULL))

    in_maps = []
    for core in range(N_CORES):
        o0 = core * NODES_PER_CORE
        m_core = mapping[o0 : o0 + NODES_PER_CORE]  # [256, 6]
        m3 = m_core.reshape(NCHUNK, P, NB)  # [cc, o_p, j]
        # slot s holds wire j = 5-s; row index in G = wire id directly
        tvals = np.transpose(m3[:, :, ::-1], (0, 2, 1)).reshape(-1)  # (cc, s, o_p)
        gidx16 = tvals.reshape(-1, 16).T.astype(np.int16)  # [16, cc*48]
        gidx_arr = np.ascontiguousarray(np.tile(gidx16, (P // 16, 1)))

        lut_core = lut_table[o0 : o0 + NODES_PER_CORE]
        lutg_arr = np.ascontiguousarray(
            lut_core.reshape(NCHUNK, P, 64).transpose(1, 0, 2)
        )
        in_maps.append({"xt": xt_arr, "gidx": gidx_arr, "lutg": lutg_arr})
    return in_maps


def assemble_output(results):
    """results: 8 dicts with 'outs' [128, 2, 1024] -> full [1024, 2048]."""
    out = np.empty((B_FULL, OUT), dtype=np.float32)
    for core in range(N_CORES):
        arr = results[core]["outs"]  # [o_p, cc, b]
        for cc in range(NCHUNK):
            o0 = core * NODES_PER_CORE + cc * P
            out[:, o0 : o0 + P] = arr[:, cc, :].T
    return out


def kernel_with_results(x, lut_table, mapping, **kwargs):
    nc = _program()
    in_maps = make_inputs(x, lut_table, mapping)
    res = run_bass_kernel_spmd(nc, in_maps, core_ids=list(range(N_CORES)), **kwargs)
    return assemble_output(res.results), res


def kernel(x, lut_table, mapping):
    out, _ = kernel_with_results(x, lut_table, mapping)
    return out


if __name__ == "__main__":
    rng = np.random.default_rng(0)
    x = rng.random((B_FULL, IN), dtype=np.float32)
    lut = rng.standard_normal((OUT, 64), dtype=np.float32)
    mp = rng.integers(0, IN, (OUT, NB), dtype=np.int32)
    out = kernel(x, lut, mp)
    print(out.shape, out.dtype)
